# revision 10
# baseline (speedup 1.0000x reference)
"""Additive attention (Bahdanau) Trainium2 kernel, 8-core data parallel.

out = softmax_k(mask(sum_h w_v[h] * tanh(q@Wq [q,h] + k@Wk [k,h]))) @ V

Per-core work (2 batches): dominated by tanh over 2*64*512*256 = 16.8M
elements on the Scalar (ACT) engine -> ~110us floor.  Pipeline:
  DVE:  sum slab  s[h, q, k] = kfT[h,k] + qfT[h,q]   (bf16 tensor_scalar)
  ACT:  tanh over big slabs (several q's per instruction)
  PE :  score rows via accumulating matmuls with one-hot weights
        lhsT_q = w_v (x) e_q (padded to M=128 for fast weight load)
        -> psum[q, :] += w_v . tanh_feat_q
  then masked exp (bias rows from valid_lens fused into psum evacuation),
  PE-transpose of unnormalized exp, exp.T stationary @ V, then row
  normalization folded into the psum evacuation of the output.

Startup latency is minimized by issuing critical-path DMAs (keys blocks)
first on their own queue and pipelining key-block transposes with the
remaining loads; compile-time constants (identity, one-hot diagonal,
k-index row) are inlined into the NEFF.
"""

import os
from contextlib import ExitStack

import ml_dtypes
import numpy as np

import concourse.bacc as bacc
import concourse.bass as bass
import concourse.mybir as mybir
import concourse.tile as tile
from concourse.bass_utils import run_bass_kernel_spmd

F32 = mybir.dt.float32
BF16 = mybir.dt.bfloat16
I32 = mybir.dt.int32
AF = mybir.ActivationFunctionType
ALU = mybir.AluOpType

B, NQ, NK, QS, KS, H, VD = 16, 64, 512, 256, 256, 256, 256
NCORES = 8
BPC = B // NCORES  # batches per core
MASK_NEG = -30.0  # exp(-30+5) ~ 1e-11 of any valid term; scores are in [-5, 5]

# chunk sizes (queries per tanh slab); first batch ramps up so the first
# ACT instruction issues as early as possible
CHUNKS_B0 = [4, 4] + [8] * 7
CHUNKS_B1 = [8] * 8


def _build():
    nc = bacc.Bacc()
    q_d = nc.declare_dram_parameter("queries", [BPC, NQ, QS], F32, isOutput=False)
    k_d = nc.declare_dram_parameter("keys", [BPC, NK, KS], F32, isOutput=False)
    v_d = nc.declare_dram_parameter("values", [BPC, NK, VD], F32, isOutput=False)
    vl_d = nc.declare_dram_parameter("valid_lens", [BPC, 1], I32, isOutput=False)
    wq_d = nc.declare_dram_parameter("W_q", [QS, H], F32, isOutput=False)
    wk_d = nc.declare_dram_parameter("W_k", [KS, H], F32, isOutput=False)
    wv_d = nc.declare_dram_parameter("w_v", [H], F32, isOutput=False)
    out_d = nc.declare_dram_parameter("out", [BPC, NQ, VD], F32, isOutput=True)

    # compile-time constants baked into the NEFF
    ident_d = nc.inline_tensor(np.eye(128, dtype=np.float32), name="ident_c")
    identb_d = nc.inline_tensor(
        np.eye(128).astype(ml_dtypes.bfloat16), name="identb_c"
    )
    # one-hot pattern padded to 128 columns (cols >= NQ are zero) so the
    # score-reduction LDWEIGHTS qualifies for fast weight load
    diag_d = nc.inline_tensor(
        np.eye(NQ, 128).astype(ml_dtypes.bfloat16).reshape(NQ * 128), name="diag_c"
    )
    krow_d = nc.inline_tensor(np.arange(NK, dtype=np.float32), name="krow_c")

    with ExitStack() as ctx:
        tc = ctx.enter_context(tile.TileContext(nc))
        consts = ctx.enter_context(tc.tile_pool(name="consts", bufs=1))
        setup = ctx.enter_context(tc.tile_pool(name="setup", bufs=2))
        slabs = ctx.enter_context(tc.tile_pool(name="slabs", bufs=2))
        sm = ctx.enter_context(tc.tile_pool(name="sm", bufs=1))
        outp = ctx.enter_context(tc.tile_pool(name="outp", bufs=2))
        ps_sc = ctx.enter_context(tc.tile_pool(name="ps_sc", bufs=2, space="PSUM"))
        ps_misc = ctx.enter_context(tc.tile_pool(name="ps_misc", bufs=2, space="PSUM"))
        ps_out = ctx.enter_context(tc.tile_pool(name="ps_out", bufs=2, space="PSUM"))

        # ---------------- loads ----------------
        # critical path first: batch-0 keys blocks on the sync queue
        k_sbs = []
        for b in range(BPC):
            k_sbs.append(
                setup.tile([128, 4, KS], F32, tag=f"k_sb{b}", name=f"k_sb{b}")
            )
        for kb in range(4):
            nc.sync.dma_start(
                out=k_sbs[0][:, kb],
                in_=k_d[0].rearrange("(kb p) d -> p kb d", p=128)[:, kb],
            )
        q_sb0 = setup.tile([NQ, QS], F32, tag="q_sb0")
        nc.sync.dma_start(out=q_sb0, in_=q_d[0])

        # everything else on the gpsimd queue
        ident = consts.tile([128, 128], F32)
        nc.gpsimd.dma_start(out=ident, in_=ident_d[:, :])
        wk_sb = setup.tile([128, 2, H], F32, tag="wk_f")
        nc.gpsimd.dma_start(out=wk_sb, in_=wk_d.rearrange("(kt p) m -> p kt m", p=128))
        wq_sb = setup.tile([128, 2, H], F32, tag="wq_f")
        nc.gpsimd.dma_start(out=wq_sb, in_=wq_d.rearrange("(kt p) m -> p kt m", p=128))
        wv_row = setup.tile([1, H], F32, tag="wv_row")
        nc.gpsimd.dma_start(out=wv_row, in_=wv_d[None, :])
        identb = consts.tile([128, 128], BF16)
        nc.gpsimd.dma_start(out=identb, in_=identb_d[:, :])
        diag_bf = consts.tile([128, NQ, 128], BF16)
        nc.gpsimd.dma_start(out=diag_bf, in_=diag_d[None, :].partition_broadcast(128))
        krow = consts.tile([NQ, NK], F32)
        nc.gpsimd.dma_start(out=krow, in_=krow_d[None, :].partition_broadcast(NQ))

        wq_bf = consts.tile([128, 2, H], BF16)
        wk_bf = consts.tile([128, 2, H], BF16)
        for kt in range(2):
            nc.vector.tensor_copy(out=wk_bf[:, kt], in_=wk_sb[:, kt])
            nc.vector.tensor_copy(out=wq_bf[:, kt], in_=wq_sb[:, kt])

        # w_v -> wv_col[p, ht] via PE transpose of a [1, 128] row
        wv_col = consts.tile([128, 2], F32)
        for ht in range(2):
            pst = ps_misc.tile([128, 512], F32, tag="ps_misc")
            nc.tensor.transpose(
                pst[:, 0:1], wv_row[0:1, ht * 128 : (ht + 1) * 128], ident[0:1, 0:1]
            )
            nc.vector.tensor_copy(out=wv_col[:, ht : ht + 1], in_=pst[:, 0:1])

        # one-hot reduction weights: oh[ht][p, q, c] = w_v[ht*128+p]*(q==c), c<128
        onehot = consts.tile([128, 2, NQ, 128], BF16)
        for ht in range(2):
            nc.vector.tensor_scalar_mul(
                out=onehot[:, ht], in0=diag_bf, scalar1=wv_col[:, ht : ht + 1]
            )

        # ---------------- per-batch compute ----------------
        for b in range(BPC):
            k_sb = k_sbs[b]
            if b == 0:
                q_sb = q_sb0
            else:
                # load batch-1 tensors (overlaps batch-0 compute)
                for kb in range(4):
                    nc.sync.dma_start(
                        out=k_sb[:, kb],
                        in_=k_d[b].rearrange("(kb p) d -> p kb d", p=128)[:, kb],
                    )
                q_sb = setup.tile([NQ, QS], F32, tag="q_sb1")
                nc.sync.dma_start(out=q_sb, in_=q_d[b])
            v_sb = setup.tile([128, 4, VD], F32, tag="v_sb")
            nc.gpsimd.dma_start(
                out=v_sb, in_=v_d[b].rearrange("(kb p) d -> p kb d", p=128)
            )
            v_bf = setup.tile([128, 4, VD], BF16, tag="v_bf")
            for kb in range(4):
                nc.vector.tensor_copy(out=v_bf[:, kb], in_=v_sb[:, kb])

            # transpose keys (cast to bf16): kT[p, kt, kb*128+j]
            kT_bf = setup.tile([128, 2, NK], BF16, tag="kT")
            for kb in range(4):
                for kt in range(2):
                    pst = ps_misc.tile([128, 512], F32, tag="ps_misc")
                    nc.tensor.transpose(
                        pst[:, 0:128], k_sb[:, kb, kt * 128 : (kt + 1) * 128], ident
                    )
                    nc.vector.tensor_copy(
                        out=kT_bf[:, kt, kb * 128 : (kb + 1) * 128], in_=pst[:, 0:128]
                    )
            # transpose queries
            qT_bf = setup.tile([128, 2, NQ], BF16, tag="qT")
            for kt in range(2):
                pst = ps_misc.tile([128, 512], F32, tag="ps_misc")
                nc.tensor.transpose(
                    pst[:, 0:NQ], q_sb[:, kt * 128 : (kt + 1) * 128], ident[0:NQ, 0:NQ]
                )
                nc.vector.tensor_copy(out=qT_bf[:, kt, :], in_=pst[:, 0:NQ])

            # projections (bf16 matmuls) -> kfT [h, k] bf16, qfT [h, q] f32
            kfT_bf = setup.tile([128, 2, NK], BF16, tag="kfT")
            for mt in range(2):
                psp = ps_misc.tile([128, 512], F32, tag="ps_misc")
                for kt in range(2):
                    nc.tensor.matmul(
                        psp,
                        lhsT=wk_bf[:, kt, mt * 128 : (mt + 1) * 128],
                        rhs=kT_bf[:, kt, :],
                        start=(kt == 0),
                        stop=(kt == 1),
                    )
                nc.vector.tensor_copy(out=kfT_bf[:, mt], in_=psp)
            qfT_f32 = setup.tile([128, 2, NQ], F32, tag="qfTf")
            for mt in range(2):
                psp = ps_misc.tile([128, 512], F32, tag="ps_misc")
                for kt in range(2):
                    nc.tensor.matmul(
                        psp[:, 0:NQ],
                        lhsT=wq_bf[:, kt, mt * 128 : (mt + 1) * 128],
                        rhs=qT_bf[:, kt, :],
                        start=(kt == 0),
                        stop=(kt == 1),
                    )
                nc.vector.tensor_copy(out=qfT_f32[:, mt], in_=psp[:, 0:NQ])

            # mask bias rows: bias[q, k] = 0 if k < valid else MASK_NEG
            valid_sb = setup.tile([NQ, 1], I32, tag="valid")
            nc.gpsimd.dma_start(
                out=valid_sb, in_=vl_d[b : b + 1, :].partition_broadcast(NQ)
            )
            valid_f = setup.tile([NQ, 1], F32, tag="validf")
            nc.vector.tensor_copy(out=valid_f, in_=valid_sb)
            bias_b = setup.tile([NQ, NK], F32, tag="bias")
            nc.vector.tensor_scalar(
                out=bias_b, in0=krow, scalar1=valid_f[:, 0:1], scalar2=None,
                op0=ALU.is_lt,
            )
            nc.vector.tensor_scalar(
                out=bias_b, in0=bias_b, scalar1=1.0, scalar2=-MASK_NEG,
                op0=ALU.subtract, op1=ALU.mult,
            )

            # ---- main loop: features + score reduction ----
            chunks = CHUNKS_B0 if b == 0 else CHUNKS_B1
            sc_ps = ps_sc.tile([128, NK], F32, tag="sc")
            q0 = 0
            first = True
            for ci, qn in enumerate(chunks):
                last_chunk = ci == len(chunks) - 1
                feat = slabs.tile([128, 8, 2, NK], BF16, tag="feat")
                sum_bf = slabs.tile([128, 8, 2, NK], BF16, tag="sum")
                for qi in range(qn):
                    q = q0 + qi
                    for ht in range(2):
                        nc.vector.tensor_scalar_add(
                            out=sum_bf[:, qi, ht],
                            in0=kfT_bf[:, ht],
                            scalar1=qfT_f32[:, ht, q : q + 1],
                        )
                nc.scalar.activation(
                    out=feat[:, 0:qn], in_=sum_bf[:, 0:qn], func=AF.Tanh
                )
                for qi in range(qn):
                    q = q0 + qi
                    for ht in range(2):
                        nc.tensor.matmul(
                            sc_ps,
                            lhsT=onehot[:, ht, q],
                            rhs=feat[:, qi, ht],
                            start=first,
                            stop=(last_chunk and qi == qn - 1 and ht == 1),
                        )
                        first = False
                q0 += qn

            # ---- softmax + output (overlaps next batch) ----
            sc_sb = sm.tile([NQ, NK], F32, tag=f"scsb{b}")
            nc.vector.tensor_tensor(
                out=sc_sb, in0=sc_ps[0:NQ], in1=bias_b, op=ALU.add
            )

            e_bf = sm.tile([NQ, NK], BF16, tag=f"e{b}")
            denom = sm.tile([NQ, 1], F32, tag=f"den{b}")
            nc.scalar.activation(out=e_bf, in_=sc_sb, func=AF.Exp, accum_out=denom)
            recip = sm.tile([NQ, 1], F32, tag=f"rec{b}")
            nc.vector.reciprocal(recip, denom)

            attnT = outp.tile([128, 4, NQ], BF16, tag="attnT")
            for kb in range(4):
                pst = ps_misc.tile([128, 512], BF16, tag="ps_misc_b")
                nc.tensor.transpose(
                    pst[:, 0:NQ],
                    e_bf[:, kb * 128 : (kb + 1) * 128],
                    identb[0:NQ, 0:NQ],
                )
                nc.vector.tensor_copy(out=attnT[:, kb], in_=pst[:, 0:NQ])

            po = ps_out.tile([NQ, VD], F32, tag="po")
            for kb in range(4):
                nc.tensor.matmul(
                    po,
                    lhsT=attnT[:, kb],
                    rhs=v_bf[:, kb],
                    start=(kb == 0),
                    stop=(kb == 3),
                )
            o_sb = outp.tile([NQ, VD], F32, tag="o_sb")
            nc.vector.tensor_scalar_mul(out=o_sb, in0=po, scalar1=recip[:, 0:1])
            nc.sync.dma_start(out=out_d[b], in_=o_sb)

    nc.compile()
    return nc


_NC_CACHE = None
LAST_RESULTS = None


def kernel(queries, keys, values, valid_lens, W_q, W_k, w_v):
    global _NC_CACHE, LAST_RESULTS
    if _NC_CACHE is None:
        _NC_CACHE = _build()
    nc = _NC_CACHE

    queries = np.ascontiguousarray(queries, dtype=np.float32)
    keys = np.ascontiguousarray(keys, dtype=np.float32)
    values = np.ascontiguousarray(values, dtype=np.float32)
    valid_lens = np.ascontiguousarray(valid_lens, dtype=np.int32)
    W_q = np.ascontiguousarray(W_q, dtype=np.float32)
    W_k = np.ascontiguousarray(W_k, dtype=np.float32)
    w_v = np.ascontiguousarray(w_v, dtype=np.float32)

    in_maps = []
    for c in range(NCORES):
        lo, hi = c * BPC, (c + 1) * BPC
        in_maps.append(
            {
                "queries": queries[lo:hi],
                "keys": keys[lo:hi],
                "values": values[lo:hi],
                "valid_lens": valid_lens[lo:hi].reshape(BPC, 1),
                "W_q": W_q,
                "W_k": W_k,
                "w_v": w_v,
            }
        )

    trace = os.environ.get("ATTN_TRACE", "0") == "1"
    res = run_bass_kernel_spmd(
        nc, in_maps, core_ids=list(range(NCORES)), trace=trace
    )
    LAST_RESULTS = res
    return np.concatenate([r["out"] for r in res.results], axis=0)
